# revision 3
# baseline (speedup 1.0000x reference)
"""NT-Xent (SimCLR) contrastive loss on 8 Trainium2 NeuronCores.

Strategy (fully SPMD, no collectives):
  z = normalize(concat(emb_i, emb_j))  # [8192, 512]
  Each core c handles a 1024-row block of z. Inputs are pre-rotated on the
  host (np.roll by -c*1024 rows) so every core runs the identical program on
  rows 0..1023 of its own rotated copy: positive pair of rotated row i is
  rotated row (i + 4096) % 8192 for every core.

  Per core:
    - normalize all 8192 rows (fp32 norms, bf16 z), building zT [512, 8192]
      (d on partitions) via SBUF->SBUF DMA xbar transposes
    - sim row-block = zT[:, :1024].T @ zT  as 128x512 PSUM tiles (bf16 matmul)
    - ACT computes exp(2*sim) with free-dim accumulation -> row denominators
      (the exp matrix is never materialized)
    - self-dot and positive-pair dot per row via fused DVE multiply+reduce
    - loss_row = ln(denom - exp(2*selfdot)) - 2*posdot
  Host: gather 8x1024 row losses, mean.
"""

import numpy as np

import concourse.bacc as bacc
import concourse.tile as tile
from concourse import mybir
from concourse.bass_utils import run_bass_kernel_spmd

N_CORES = 8
D = 512
ROWS = 8192
BLK = ROWS // N_CORES  # 1024
P = 128
N_CHUNKS = ROWS // P  # 64 row-chunks of 128
BLK_CHUNKS = BLK // P  # 8
GROUP = 8  # chunks per norm batch (one Ln/Exp pair per group)
NT = 512  # matmul moving free dim (one PSUM bank of fp32)
N_NT = ROWS // NT  # 16
KD = D // P  # 4 contraction tiles

f32 = mybir.dt.float32
bf16 = mybir.dt.bfloat16


def _build():
    Alu = mybir.AluOpType
    Act = mybir.ActivationFunctionType

    nc = bacc.Bacc("TRN2", target_bir_lowering=False)
    emb = nc.dram_tensor("emb", [ROWS, D], f32, kind="ExternalInput")
    loss = nc.dram_tensor("loss", [P, BLK_CHUNKS], f32, kind="ExternalOutput")

    with tile.TileContext(nc) as tc:
        with (
            tc.tile_pool(name="persist", bufs=1) as persist,
            tc.tile_pool(name="loads", bufs=18) as loads,
            tc.tile_pool(name="zbs", bufs=6) as zbs,
            tc.tile_pool(name="scratch", bufs=3) as scratch,
            tc.tile_pool(name="small", bufs=2) as small,
            tc.tile_pool(name="psum", bufs=8, space="PSUM") as psum_pool,
        ):
            # persistent tensors
            zT = [persist.tile([P, ROWS], bf16, tag=f"zT{k}", name=f"zT{k}") for k in range(KD)]
            acc = [
                persist.tile([P, N_NT], f32, tag=f"acc{m}", name=f"acc{m}") for m in range(BLK_CHUNKS)
            ]
            selfd = persist.tile([P, BLK_CHUNKS], f32, tag="selfd")
            posd = persist.tile([P, BLK_CHUNKS], f32, tag="posd")

            # blk chunks first (feed lhsT + selfdot), then pos chunks, then rest
            chunk_order = (
                list(range(0, BLK_CHUNKS))
                + list(range(32, 32 + BLK_CHUNKS))
                + list(range(BLK_CHUNKS, 32))
                + list(range(32 + BLK_CHUNKS, N_CHUNKS))
            )
            zb_keep = {}
            for gi in range(0, N_CHUNKS, GROUP):
                grp = chunk_order[gi : gi + GROUP]
                sq = small.tile([P, GROUP], f32, tag="sq")
                ets = []
                for idx, j in enumerate(grp):
                    et = loads.tile([P, D], f32, tag="et")
                    nc.sync.dma_start(out=et, in_=emb[j * P : (j + 1) * P, :])
                    tt = scratch.tile([P, D], bf16, tag="ttout")
                    nc.vector.scalar_tensor_tensor(
                        out=tt,
                        in0=et,
                        scalar=1.0,
                        in1=et,
                        op0=Alu.mult,
                        op1=Alu.mult,
                        accum_out=sq[:, idx : idx + 1],
                    )
                    ets.append(et)
                # 1/sqrt(x) = exp(-0.5 * ln(x)) -- keeps ACT on one table set
                lnv = small.tile([P, GROUP], f32, tag="lnv")
                nc.scalar.activation(out=lnv, in_=sq, func=Act.Ln)
                rinv = small.tile([P, GROUP], f32, tag="rinv")
                nc.scalar.activation(out=rinv, in_=lnv, func=Act.Exp, scale=-0.5)
                for idx, j in enumerate(grp):
                    keep = j < BLK_CHUNKS or 32 <= j < 32 + BLK_CHUNKS
                    if keep:
                        zb = persist.tile([P, D], bf16, tag=f"zbk{j}", name=f"zbk{j}")
                        zb_keep[j] = zb
                    else:
                        zb = zbs.tile([P, D], bf16, tag="zb")
                    nc.vector.tensor_scalar_mul(
                        out=zb, in0=ets[idx], scalar1=rinv[:, idx : idx + 1]
                    )
                    for k in range(KD):
                        nc.sync.dma_start(
                            out=zT[k][:, j * P : (j + 1) * P],
                            in_=zb[:, k * P : (k + 1) * P],
                            transpose=True,
                        )
                if gi == GROUP:
                    # blk + pos chunks all normalized: per-row self/pos dots
                    for m in range(BLK_CHUNKS):
                        t1 = scratch.tile([P, D], bf16, tag="ttout")
                        nc.vector.scalar_tensor_tensor(
                            out=t1,
                            in0=zb_keep[m],
                            scalar=1.0,
                            in1=zb_keep[m],
                            op0=Alu.mult,
                            op1=Alu.mult,
                            accum_out=selfd[:, m : m + 1],
                        )
                        t2 = scratch.tile([P, D], bf16, tag="ttout")
                        nc.vector.scalar_tensor_tensor(
                            out=t2,
                            in0=zb_keep[m],
                            scalar=1.0,
                            in1=zb_keep[32 + m],
                            op0=Alu.mult,
                            op1=Alu.mult,
                            accum_out=posd[:, m : m + 1],
                        )

            # main loop: sim tiles -> exp -> row sums, n ordered by readiness
            n_order = [0, 1, 8, 9] + list(range(2, 8)) + list(range(10, 16))
            for n in n_order:
                for m in range(BLK_CHUNKS):
                    ps = psum_pool.tile([P, NT], f32, tag="ps")
                    for k in range(KD):
                        nc.tensor.matmul(
                            ps,
                            zT[k][:, m * P : (m + 1) * P],
                            zT[k][:, n * NT : (n + 1) * NT],
                            start=(k == 0),
                            stop=(k == KD - 1),
                        )
                    ex = scratch.tile([P, NT], bf16, tag="exout")
                    nc.scalar.activation(
                        out=ex,
                        in_=ps,
                        func=Act.Exp,
                        scale=2.0,
                        accum_out=acc[m][:, n : n + 1],
                    )

            # finale: loss_row = ln(denom - exp(2*selfdot)) - 2*posdot
            dsum = persist.tile([P, BLK_CHUNKS], f32, tag="dsum")
            for m in range(BLK_CHUNKS):
                nc.vector.reduce_sum(
                    out=dsum[:, m : m + 1], in_=acc[m], axis=mybir.AxisListType.X
                )
            sexp = small.tile([P, BLK_CHUNKS], f32, tag="sexp")
            nc.scalar.activation(out=sexp, in_=selfd, func=Act.Exp, scale=2.0)
            dx = small.tile([P, BLK_CHUNKS], f32, tag="dx")
            nc.vector.tensor_sub(dx, dsum, sexp)
            ld = small.tile([P, BLK_CHUNKS], f32, tag="ld")
            nc.scalar.activation(out=ld, in_=dx, func=Act.Ln)
            lossv = small.tile([P, BLK_CHUNKS], f32, tag="lossv")
            nc.vector.scalar_tensor_tensor(
                out=lossv,
                in0=posd,
                scalar=-2.0,
                in1=ld,
                op0=Alu.mult,
                op1=Alu.add,
            )
            nc.sync.dma_start(out=loss[:, :], in_=lossv)

    nc.compile()
    return nc


_NC_CACHE = []


def _get_nc():
    if not _NC_CACHE:
        _NC_CACHE.append(_build())
    return _NC_CACHE[0]


def make_in_maps(emb_i: np.ndarray, emb_j: np.ndarray):
    emb_all = np.concatenate(
        [np.asarray(emb_i, np.float32), np.asarray(emb_j, np.float32)], axis=0
    )
    return [
        {"emb": np.ascontiguousarray(np.roll(emb_all, -c * BLK, axis=0))}
        for c in range(N_CORES)
    ]


def assemble(results) -> np.ndarray:
    rows = []
    for c in range(N_CORES):
        out = results[c]["loss"]  # [128, 8]; out[p, m] = loss of block row m*128+p
        rows.append(out.T.reshape(-1))
    all_rows = np.concatenate(rows)  # original row order
    return np.float32(all_rows.astype(np.float64).mean())


def kernel(emb_i: np.ndarray, emb_j: np.ndarray) -> np.ndarray:
    nc = _get_nc()
    res = run_bass_kernel_spmd(nc, make_in_maps(emb_i, emb_j), core_ids=list(range(N_CORES)))
    return assemble(res.results)


if __name__ == "__main__":
    rng = np.random.default_rng(0)
    ei = rng.standard_normal((4096, D)).astype(np.float32)
    ej = rng.standard_normal((4096, D)).astype(np.float32)
    print(kernel(ei, ej))
